# revision 1
# baseline (speedup 1.0000x reference)
"""Trainium2 Bass kernel for the ContinuousVariableQNN problem (v2).

Math reduction (validated against the jax reference on host):
  The reference builds a 256x256 symplectic matrix S from params, then
    mu   = mu0 @ S.T   with mu0[:, 0::2] = 2*inputs (odd cols zero)
    n    = (dsum + mu_x^2 + mu_p^2) / (2*hbar) - 0.5
  Because mu0's p-quadrature entries are all zero, the big matmul collapses to
    mu_dev = inputs @ Ms          with Ms[i, j] = S[j, 2*i]   ([128, 256])
  (factor 2 from displacement and the 1/4 normalization cancel), and
    n[b, m] = mu_dev[b, 2m]^2 + mu_dev[b, 2m+1]^2 + bias[m]
  with bias[m] = (diag(S S^T)[2m] + diag(S S^T)[2m+1])/4 - 0.5 (a constant).

v2 device strategy (transposed orientation, fp16 operands, bf16 output):
  Host pre-transposes X so tiles arrive as X^T [feature, batch]: no PE
  transposes at all.  Per 512-batch-column chunk, two stationary-weight
  matmuls (Mx = Ms[:, 0::2], Mp = Ms[:, 1::2], both fp16):
      mux^T = Mx^T @ X^T-chunk   -> PSUM bank   [mode, batch]
      mup^T = Mp^T @ X^T-chunk   -> PSUM bank
  Mode index lands on partitions, so bias is a per-partition scalar and no
  stride-2 de-interleave is needed.  Tail per chunk pair:
      ACT   : sqx = Square(mux^T)              (PSUM f32 -> SBUF bf16)
      DVE   : sqp = mup^T * mup^T              (PSUM f32 -> SBUF bf16)
      DVE/GPSIMD (alternating): out = (sqx + bias) + sqp   (one
              scalar_tensor_tensor, all-bf16 SBUF -> DVE 4x mode)
  IO is fp16 in / bf16 out, halving HBM traffic vs f32 (DMA floor ~23us/core).
  Host-simulated pipeline rel err vs f64 reference: 8.0e-3 (gate 2e-2).
  Input pieces ride the SP HWDGE queue, output pieces the ACT HWDGE queue,
  4KB contiguous per partition per piece.
"""

import ml_dtypes
import numpy as np

import concourse.bass as bass
import concourse.mybir as mybir
import concourse.tile as tile
from concourse import bacc
from concourse.bass_utils import run_bass_kernel_spmd

N_QUMODES = 128
N_LAYERS = 8
BATCH = 131072
N_CORES = 8
ROWS = BATCH // N_CORES          # 16384 batch columns per core (free dim)
PIECES = 8                       # DMA granularity: 2048 cols = 4KB/partition
PC = ROWS // PIECES              # 2048
CHUNK = 512                      # matmul free dim (one PSUM bank of f32)
PAIRS = ROWS // (2 * CHUNK)      # 16 chunk pairs
F32 = mybir.dt.float32
F16 = mybir.dt.float16
BF16 = mybir.dt.bfloat16


def host_prep(params: np.ndarray):
    """Build Mxp [128, 256] fp16 (Mx | Mp) and bias [128, 1] f32 on host."""
    L, N = N_LAYERS, N_QUMODES
    p = params.reshape(L, N, 3).astype(np.float64)
    th1, r, th2 = p[..., 0], p[..., 1], p[..., 2]

    def rot(th):
        c, s = np.cos(th), np.sin(th)
        return np.stack([np.stack([c, -s], -1), np.stack([s, c], -1)], -2)

    z = np.zeros_like(r)
    sq = np.stack([np.stack([np.exp(-r), z], -1),
                   np.stack([z, np.exp(r)], -1)], -2)
    blk = np.einsum('lnab,lnbc,lncd->lnad', rot(th2), sq, rot(th1))

    t = np.cos(np.pi / 4)
    rr = np.sin(np.pi / 4)
    BS4 = np.array([[t, 0., -rr, 0.],
                    [0., t, 0., -rr],
                    [rr, 0., t, 0.],
                    [0., rr, 0., t]])
    C = np.eye(2 * N)
    for i in range(N - 1):
        C[2 * i:2 * i + 4, :] = BS4 @ C[2 * i:2 * i + 4, :]

    S = np.eye(2 * N)
    idx = np.arange(N)
    for l in range(L):
        D = np.zeros((N, 2, N, 2))
        D[idx, :, idx, :] = blk[l]
        S = C @ (D.reshape(2 * N, 2 * N) @ S)

    # mu_dev[b, j] = (inputs @ Ms)[b, j] with Ms = S[:, 0::2].T  [128, 256].
    Ms = S[:, 0::2].T
    Mx = Ms[:, 0::2]                 # [128 feat, 128 mode] x-quadrature
    Mp = Ms[:, 1::2]                 # p-quadrature
    mxp = np.ascontiguousarray(
        np.concatenate([Mx, Mp], axis=1)).astype(np.float16)   # [128, 256]

    dV = (S ** 2).sum(axis=1)                                  # [256]
    bias = ((dV[0::2] + dV[1::2]) / 4.0 - 0.5)
    bias_col = np.ascontiguousarray(bias.reshape(128, 1)).astype(np.float32)
    return mxp, bias_col


def build_bass():
    nc = bacc.Bacc("TRN2", target_bir_lowering=False, debug=False,
                   num_devices=N_CORES)

    x_d = nc.dram_tensor("x", [128, ROWS], F16, kind="ExternalInput")
    mxp_d = nc.dram_tensor("mxp", [128, 256], F16, kind="ExternalInput")
    bias_d = nc.dram_tensor("bias", [128, 1], F32, kind="ExternalInput")
    out_d = nc.dram_tensor("out", [128, ROWS], BF16, kind="ExternalOutput")

    x_v = x_d.ap().rearrange("p (k c) -> k p c", c=PC)     # [8][128, 2048]
    out_v = out_d.ap().rearrange("p (k c) -> k p c", c=PC)

    with tile.TileContext(nc) as tc:
        with (
            tc.tile_pool(name="const", bufs=1) as const_pool,
            tc.tile_pool(name="xin", bufs=PIECES) as xin_pool,
            tc.tile_pool(name="oout", bufs=5) as oout_pool,
            tc.tile_pool(name="sq", bufs=8) as sq_pool,
            tc.tile_pool(name="mup", bufs=2, space="PSUM") as mup_pool,
        ):
            mxp_sb = const_pool.tile([128, 256], F16)
            nc.sync.dma_start(out=mxp_sb, in_=mxp_d.ap())
            bias_sb = const_pool.tile([128, 1], F32)
            nc.sync.dma_start(out=bias_sb, in_=bias_d.ap())

            x_tiles = []
            for k in range(PIECES):
                x_sb = xin_pool.tile([128, PC], F16, tag="x_sb",
                                     name=f"x_sb_{k}")
                if k == 0:
                    # halve the first transfer so the PE can start sooner
                    nc.sync.dma_start(out=x_sb[:, 0:PC // 2],
                                      in_=x_v[k][:, 0:PC // 2])
                    nc.sync.dma_start(out=x_sb[:, PC // 2:],
                                      in_=x_v[k][:, PC // 2:])
                else:
                    nc.sync.dma_start(out=x_sb, in_=x_v[k])
                x_tiles.append(x_sb)

            out_tiles = {}
            for g in range(PAIRS):
                k, gk = divmod(g, PAIRS // PIECES)    # piece idx, pair in piece
                if gk == 0:
                    out_tiles[k] = oout_pool.tile([128, 4, CHUNK], BF16,
                                                  tag="o_sb", name=f"o_sb_{k}")
                x_sb = x_tiles[k]
                c0 = 2 * gk * CHUNK                   # col offset in piece
                c1 = c0 + CHUNK

                # 4 matmuls per pair (512 f32 = one PSUM bank is the ISA
                # max): banks [mux0, mux1, mup0, mup1], same-weight matmuls
                # adjacent to minimize PE weight reloads.
                ps = mup_pool.tile([128, 4, CHUNK], F32)
                nc.tensor.matmul(ps[:, 0, :], mxp_sb[:, 0:128],
                                 x_sb[:, c0:c0 + CHUNK], start=True, stop=True)
                nc.tensor.matmul(ps[:, 1, :], mxp_sb[:, 0:128],
                                 x_sb[:, c1:c1 + CHUNK], start=True, stop=True)
                nc.tensor.matmul(ps[:, 2, :], mxp_sb[:, 128:256],
                                 x_sb[:, c0:c0 + CHUNK], start=True, stop=True)
                nc.tensor.matmul(ps[:, 3, :], mxp_sb[:, 128:256],
                                 x_sb[:, c1:c1 + CHUNK], start=True, stop=True)

                # ONE ACT pass squares all 4 banks (single PSUM input AP),
                # then ONE DVE fused combine per pair.  Minimizing the
                # instruction count keeps the engine queues off the
                # semaphore-processing floor and lets the power governor
                # relax (fewer concurrently-hot engines).
                sq = sq_pool.tile([128, 4, CHUNK], BF16, tag="sq",
                                  name=f"sq_{g}")
                nc.scalar.activation(sq, ps,
                                     mybir.ActivationFunctionType.Square)
                o_sb = out_tiles[k]
                nc.vector.scalar_tensor_tensor(
                    out=o_sb[:, 2 * gk:2 * gk + 2, :], in0=sq[:, 0:2, :],
                    scalar=bias_sb, in1=sq[:, 2:4, :],
                    op0=mybir.AluOpType.add, op1=mybir.AluOpType.add)

                if gk == PAIRS // PIECES - 1:
                    # Outputs ride the SP queue too: in+out serialize there
                    # at exactly the aggregate DMA floor, and the ACT
                    # sequencer keeps all its time for the squares.  The
                    # last piece goes out in two halves so the final
                    # transfer (the drain tail) is half as long.
                    o_flat = out_tiles.pop(k).rearrange("p a b -> p (a b)")
                    if k == PIECES - 1:
                        nc.sync.dma_start(out=out_v[k][:, 0:PC // 2],
                                          in_=o_flat[:, 0:PC // 2])
                        nc.sync.dma_start(out=out_v[k][:, PC // 2:],
                                          in_=o_flat[:, PC // 2:])
                    else:
                        nc.sync.dma_start(out=out_v[k], in_=o_flat)

    nc.compile()
    return nc


_NC_CACHE = None


def make_in_maps(X: np.ndarray, params: np.ndarray):
    mxp, bias_col = host_prep(params)
    xt = np.ascontiguousarray(X.astype(np.float16).T)     # [128, BATCH]
    return [
        {"x": np.ascontiguousarray(xt[:, i * ROWS:(i + 1) * ROWS]),
         "mxp": mxp, "bias": bias_col}
        for i in range(N_CORES)
    ]


def assemble_output(results) -> np.ndarray:
    full = np.concatenate([r["out"] for r in results], axis=1)  # [128, BATCH]
    return np.ascontiguousarray(full.T.astype(np.float32))


def kernel(**inputs: np.ndarray) -> np.ndarray:
    global _NC_CACHE
    X = np.asarray(inputs["inputs"], dtype=np.float32)
    params = np.asarray(inputs["params"], dtype=np.float32)
    assert X.shape == (BATCH, N_QUMODES)

    if _NC_CACHE is None:
        _NC_CACHE = build_bass()
    nc = _NC_CACHE

    in_maps = make_in_maps(X, params)
    res = run_bass_kernel_spmd(nc, in_maps, core_ids=list(range(N_CORES)))
    return assemble_output(res.results)



# revision 7
# speedup vs baseline: 1.0108x; 1.0108x over previous
"""Trainium2 Bass kernel for the ContinuousVariableQNN problem (v4).

Math reduction (validated against the jax reference on host):
  The reference builds a 256x256 symplectic matrix S from params, then
    mu   = mu0 @ S.T   with mu0[:, 0::2] = 2*inputs (odd cols zero)
    n    = (dsum + mu_x^2 + mu_p^2) / (2*hbar) - 0.5
  Because mu0's p-quadrature entries are all zero, the big matmul collapses to
    mu_dev = inputs @ Ms          with Ms[i, j] = S[j, 2*i]   ([128, 256])
  (factor 2 from displacement and the 1/4 normalization cancel), and
    n[b, m] = mu_dev[b, 2m]^2 + mu_dev[b, 2m+1]^2 + bias[m]
  with bias[m] = (diag(S S^T)[2m] + diag(S S^T)[2m+1])/4 - 0.5 (a constant).

v4 device strategy (vs the 49.4us v2 baseline, from NTFF trace analysis):
  v2's wall was the Scalar engine squaring all four PSUM banks per pair
  (2.0us x 16 = 32us busy) plus a DVE combine pass.  The ISA allows at most
  ONE PSUM input per vector instruction, so the mup^2 + sqx combine cannot
  be a stock two-PSUM op -- instead v4 registers a custom DVE microcode op
  (the documented extension point in concourse.dve_ops):
      SQUARE_PLUS_ANT:  out = sq(Src0) + Src1      (1 uop, row 17)
  Per 1024-col pair (PSUM banks [mux0 mux1 mup0 mup1]):
      ACT: sqx = Square(mux banks)      PSUM->SBUF bf16   (1024+352)/1.2GHz
      DVE: out = sq(mup banks) + sqx    PSUM+SBUF->SBUF   ~(1024+240)/0.96GHz
  Two perfectly balanced passes (~1.15us / ~1.35us), zero extra combine.
  The +bias[m] is applied on the host during unshard (f32 broadcast add
  folded into the existing transpose+astype).  IO: all input pieces trigger
  first on the SP HWDGE ring (ring FIFO = input streams at line rate,
  outputs drain behind); 1024-col lead-in/tail pieces shorten the edges.
  Expected ~ring-bound: ~23.4us of HBM bytes + fixed framework overhead.
"""

import ml_dtypes
import numpy as np

import concourse.bass as bass
import concourse.mybir as mybir
import concourse.tile as tile
from concourse import bacc
from concourse import dve_ops as _D
from concourse.dve_spec import Spec, Src0, Src1, sq as _sq, lower as _lower
from concourse.dve_uop import DveOpSpec as _DveOpSpec
from concourse.bass_utils import run_bass_kernel_spmd

N_QUMODES = 128
N_LAYERS = 8
BATCH = 131072
N_CORES = 8
ROWS = BATCH // N_CORES          # 16384 batch columns per core (free dim)
PAIRS = 16                       # 1024-col work units
F32 = mybir.dt.float32
F16 = mybir.dt.float16
BF16 = mybir.dt.bfloat16

IN_PIECES = [1024, 1024] + [2048] * 7
OUT_PIECES = [1024, 1024] + [2048] * 6 + [1024, 1024]


def _register_square_plus():
    """out = sq(Src0) + Src1 via the custom-DVE extension point."""
    name = "SQUARE_PLUS_ANT"
    if name in _D._SUB_OPCODE_FOR_NAME:
        return next(op for op in _D.OPS if op.name == name)
    spec = Spec(
        body=_sq(Src0) + Src1,
        reference=lambda in0, in1, s0, s1, imm2:
            in0.astype(np.float32) ** 2 + in1,
    )
    row = max(_D._SUB_OPCODE_FOR_NAME.values()) + 1
    assert row < 0x20
    shas = {}
    for ver in ("v3",):
        s = _DveOpSpec(name=name, opcode=row, uops=_lower(spec, ver=ver),
                       rd1_en=_D.has_src1(spec))
        shas[ver] = s.sha(ver)
    op = _D.DveOp(name, spec, subdim=False, uops_sha=shas)
    _D.OPS.append(op)
    _D.CUSTOM_DVE_SPECS[name] = spec
    _D._SUB_OPCODE_FOR_NAME[name] = row
    return op


SQUARE_PLUS = _register_square_plus()


def host_prep(params: np.ndarray):
    """Build Mxp [128, 256] fp16 (Mx | Mp) and bias [128] f64 on host."""
    L, N = N_LAYERS, N_QUMODES
    p = params.reshape(L, N, 3).astype(np.float64)
    th1, r, th2 = p[..., 0], p[..., 1], p[..., 2]

    def rot(th):
        c, s = np.cos(th), np.sin(th)
        return np.stack([np.stack([c, -s], -1), np.stack([s, c], -1)], -2)

    z = np.zeros_like(r)
    sq = np.stack([np.stack([np.exp(-r), z], -1),
                   np.stack([z, np.exp(r)], -1)], -2)
    blk = np.einsum('lnab,lnbc,lncd->lnad', rot(th2), sq, rot(th1))

    t = np.cos(np.pi / 4)
    rr = np.sin(np.pi / 4)
    BS4 = np.array([[t, 0., -rr, 0.],
                    [0., t, 0., -rr],
                    [rr, 0., t, 0.],
                    [0., rr, 0., t]])
    C = np.eye(2 * N)
    for i in range(N - 1):
        C[2 * i:2 * i + 4, :] = BS4 @ C[2 * i:2 * i + 4, :]

    S = np.eye(2 * N)
    idx = np.arange(N)
    for l in range(L):
        D = np.zeros((N, 2, N, 2))
        D[idx, :, idx, :] = blk[l]
        S = C @ (D.reshape(2 * N, 2 * N) @ S)

    # mu_dev[b, j] = (inputs @ Ms)[b, j] with Ms = S[:, 0::2].T  [128, 256].
    Ms = S[:, 0::2].T
    Mx = Ms[:, 0::2]                 # [128 feat, 128 mode] x-quadrature
    Mp = Ms[:, 1::2]                 # p-quadrature
    mxp = np.ascontiguousarray(
        np.concatenate([Mx, Mp], axis=1)).astype(np.float16)   # [128, 256]

    dV = (S ** 2).sum(axis=1)                                  # [256]
    bias = ((dV[0::2] + dV[1::2]) / 4.0 - 0.5)                 # [128] f64
    return mxp, bias


def build_bass():
    nc = bacc.Bacc("TRN2", target_bir_lowering=False, debug=False,
                   num_devices=N_CORES)

    x_d = nc.dram_tensor("x", [128, ROWS], F16, kind="ExternalInput")
    mxp_d = nc.dram_tensor("mxp", [128, 256], F16, kind="ExternalInput")
    out_d = nc.dram_tensor("out", [128, ROWS], BF16, kind="ExternalOutput")
    x_ap = x_d.ap()
    out_ap = out_d.ap()

    Sq = mybir.ActivationFunctionType.Square

    with tile.TileContext(nc) as tc:
        with (
            tc.tile_pool(name="const", bufs=1) as const_pool,
            tc.tile_pool(name="xin", bufs=len(IN_PIECES)) as xin_pool,
            tc.tile_pool(name="oout", bufs=len(OUT_PIECES)) as oout_pool,
            tc.tile_pool(name="sqx", bufs=6) as sqx_pool,
            tc.tile_pool(name="ps", bufs=2, space="PSUM") as ps_pool,
        ):
            mxp_sb = const_pool.tile([128, 256], F16)

            # Input DMAs first, in program order on the SP HWDGE ring:
            # x piece 0 -> weights -> remaining x pieces.  Ring FIFO keeps
            # input bytes ahead of all output bytes.
            x_tiles = []          # (tile, start_col, cols)
            off = 0
            for k, cols in enumerate(IN_PIECES):
                t = xin_pool.tile([128, cols], F16, tag="x_sb",
                                  name=f"x_sb_{k}")
                nc.sync.dma_start(out=t, in_=x_ap[:, off:off + cols])
                x_tiles.append((t, off, cols))
                off += cols
                if k == 0:
                    nc.sync.dma_start(out=mxp_sb, in_=mxp_d.ap())

            def locate(tiles, col):
                for t, start, cols in tiles:
                    if start <= col < start + cols:
                        return t, col - start
                raise AssertionError(col)

            o_tiles = []          # [tile, start_col, cols, pairs_done]
            ooff = 0
            for k, cols in enumerate(OUT_PIECES):
                t = oout_pool.tile([128, cols], BF16, tag="o_sb",
                                   name=f"o_sb_{k}")
                o_tiles.append([t, ooff, cols, 0])
                ooff += cols

            for g in range(PAIRS):
                cg = 1024 * g
                xt, xo = locate(x_tiles, cg)
                ps = ps_pool.tile([128, 2048], F32)
                # banks: [mux(c0) mux(c1) mup(c0) mup(c1)], 512 f32 each
                nc.tensor.matmul(ps[:, 0:512], mxp_sb[:, 0:128],
                                 xt[:, xo:xo + 512], start=True, stop=True)
                nc.tensor.matmul(ps[:, 512:1024], mxp_sb[:, 0:128],
                                 xt[:, xo + 512:xo + 1024],
                                 start=True, stop=True)
                nc.tensor.matmul(ps[:, 1024:1536], mxp_sb[:, 128:256],
                                 xt[:, xo:xo + 512], start=True, stop=True)
                nc.tensor.matmul(ps[:, 1536:2048], mxp_sb[:, 128:256],
                                 xt[:, xo + 512:xo + 1024],
                                 start=True, stop=True)

                sqx = sqx_pool.tile([128, 1024], BF16, tag="sqx",
                                    name=f"sqx_{g}")
                nc.scalar.activation(sqx, ps[:, 0:1024], Sq)

                rec = None
                for r in o_tiles:
                    if r[1] <= cg < r[1] + r[2]:
                        rec = r
                        break
                ot, ostart, ocols, _ = rec
                oo = cg - ostart
                nc.vector._custom_dve(SQUARE_PLUS, out=ot[:, oo:oo + 1024],
                                      in0=ps[:, 1024:2048], in1=sqx)

                rec[3] += 1
                if rec[3] == ocols // 1024:
                    nc.sync.dma_start(out=out_ap[:, ostart:ostart + ocols],
                                      in_=ot)

    nc.compile()
    return nc


_NC_CACHE = None
_BIAS = None


def make_in_maps(X: np.ndarray, params: np.ndarray):
    global _BIAS
    mxp, bias = host_prep(params)
    _BIAS = bias.astype(np.float32)
    xt = np.ascontiguousarray(X.astype(np.float16).T)     # [128, BATCH]
    return [
        {"x": np.ascontiguousarray(xt[:, i * ROWS:(i + 1) * ROWS]),
         "mxp": mxp}
        for i in range(N_CORES)
    ]


def assemble_output(results) -> np.ndarray:
    full = np.concatenate([r["out"] for r in results], axis=1)  # [128, BATCH]
    out = full.T.astype(np.float32)
    out += _BIAS[None, :]
    return np.ascontiguousarray(out)


def kernel(**inputs: np.ndarray) -> np.ndarray:
    global _NC_CACHE
    X = np.asarray(inputs["inputs"], dtype=np.float32)
    params = np.asarray(inputs["params"], dtype=np.float32)
    assert X.shape == (BATCH, N_QUMODES)

    if _NC_CACHE is None:
        _NC_CACHE = build_bass()
    nc = _NC_CACHE

    in_maps = make_in_maps(X, params)
    res = run_bass_kernel_spmd(nc, in_maps, core_ids=list(range(N_CORES)))
    return assemble_output(res.results)


# revision 15
# speedup vs baseline: 1.2333x; 1.2202x over previous
"""Trainium2 Bass kernel for the ContinuousVariableQNN problem (v4).

Math reduction (validated against the jax reference on host):
  The reference builds a 256x256 symplectic matrix S from params, then
    mu   = mu0 @ S.T   with mu0[:, 0::2] = 2*inputs (odd cols zero)
    n    = (dsum + mu_x^2 + mu_p^2) / (2*hbar) - 0.5
  Because mu0's p-quadrature entries are all zero, the big matmul collapses to
    mu_dev = inputs @ Ms          with Ms[i, j] = S[j, 2*i]   ([128, 256])
  (factor 2 from displacement and the 1/4 normalization cancel), and
    n[b, m] = mu_dev[b, 2m]^2 + mu_dev[b, 2m+1]^2 + bias[m]
  with bias[m] = (diag(S S^T)[2m] + diag(S S^T)[2m+1])/4 - 0.5 (a constant).

v4 device strategy (vs the 49.4us v2 baseline, from NTFF trace analysis):
  v2's wall was the Scalar engine squaring all four PSUM banks per pair
  (2.0us x 16 = 32us busy) plus a DVE combine pass.  The ISA allows at most
  ONE PSUM input per vector instruction, so the mup^2 + sqx combine cannot
  be a stock two-PSUM op -- instead v4 registers a custom DVE microcode op
  (the documented extension point in concourse.dve_ops):
      SQUARE_PLUS_ANT:  out = sq(Src0) + Src1      (1 uop, row 17)
  Per 1024-col pair (PSUM banks [mux0 mux1 mup0 mup1]):
      ACT: sqx = Square(mux banks)      PSUM->SBUF bf16   (1024+352)/1.2GHz
      DVE: out = sq(mup banks) + sqx    PSUM+SBUF->SBUF   ~(1024+240)/0.96GHz
  Two perfectly balanced passes (~1.15us / ~1.35us), zero extra combine.
  The +bias[m] is applied on the host during unshard (f32 broadcast add
  folded into the existing transpose+astype).  IO: all input pieces trigger
  first on the SP HWDGE ring (ring FIFO = input streams at line rate,
  outputs drain behind); 1024-col lead-in/tail pieces shorten the edges.
  Expected ~ring-bound: ~23.4us of HBM bytes + fixed framework overhead.
"""

import ml_dtypes
import numpy as np

import concourse.bass as bass
import concourse.mybir as mybir
import concourse.tile as tile
from concourse import bacc
from concourse import dve_ops as _D
from concourse.dve_spec import Spec, Src0, Src1, sq as _sq, lower as _lower
from concourse.dve_uop import DveOpSpec as _DveOpSpec
from concourse.bass_utils import run_bass_kernel_spmd

N_QUMODES = 128
N_LAYERS = 8
BATCH = 131072
N_CORES = 8
ROWS = BATCH // N_CORES          # 16384 batch columns per core (free dim)
PAIRS = 16                       # 1024-col work units
F32 = mybir.dt.float32
F16 = mybir.dt.float16
BF16 = mybir.dt.bfloat16

IN_PIECES = [512, 512, 1024] + [2048] * 7
OUT_PIECES = [1024, 1024] + [2048] * 6 + [1024, 512, 512]


def _register_square_plus():
    """out = sq(Src0) + Src1 via the custom-DVE extension point."""
    name = "SQUARE_PLUS_ANT"
    if name in _D._SUB_OPCODE_FOR_NAME:
        return next(op for op in _D.OPS if op.name == name)
    spec = Spec(
        body=_sq(Src0) + Src1,
        reference=lambda in0, in1, s0, s1, imm2:
            in0.astype(np.float32) ** 2 + in1,
    )
    row = max(_D._SUB_OPCODE_FOR_NAME.values()) + 1
    assert row < 0x20
    shas = {}
    for ver in ("v3",):
        s = _DveOpSpec(name=name, opcode=row, uops=_lower(spec, ver=ver),
                       rd1_en=_D.has_src1(spec))
        shas[ver] = s.sha(ver)
    op = _D.DveOp(name, spec, subdim=False, uops_sha=shas)
    _D.OPS.append(op)
    _D.CUSTOM_DVE_SPECS[name] = spec
    _D._SUB_OPCODE_FOR_NAME[name] = row
    return op


SQUARE_PLUS = _register_square_plus()


def host_prep(params: np.ndarray):
    """Build Mxp [128, 256] fp16 (Mx | Mp) and bias [128] f64 on host."""
    L, N = N_LAYERS, N_QUMODES
    p = params.reshape(L, N, 3).astype(np.float64)
    th1, r, th2 = p[..., 0], p[..., 1], p[..., 2]

    def rot(th):
        c, s = np.cos(th), np.sin(th)
        return np.stack([np.stack([c, -s], -1), np.stack([s, c], -1)], -2)

    z = np.zeros_like(r)
    sq = np.stack([np.stack([np.exp(-r), z], -1),
                   np.stack([z, np.exp(r)], -1)], -2)
    blk = np.einsum('lnab,lnbc,lncd->lnad', rot(th2), sq, rot(th1))

    t = np.cos(np.pi / 4)
    rr = np.sin(np.pi / 4)
    BS4 = np.array([[t, 0., -rr, 0.],
                    [0., t, 0., -rr],
                    [rr, 0., t, 0.],
                    [0., rr, 0., t]])
    C = np.eye(2 * N)
    for i in range(N - 1):
        C[2 * i:2 * i + 4, :] = BS4 @ C[2 * i:2 * i + 4, :]

    S = np.eye(2 * N)
    idx = np.arange(N)
    for l in range(L):
        D = np.zeros((N, 2, N, 2))
        D[idx, :, idx, :] = blk[l]
        S = C @ (D.reshape(2 * N, 2 * N) @ S)

    # mu_dev[b, j] = (inputs @ Ms)[b, j] with Ms = S[:, 0::2].T  [128, 256].
    Ms = S[:, 0::2].T
    Mx = Ms[:, 0::2]                 # [128 feat, 128 mode] x-quadrature
    Mp = Ms[:, 1::2]                 # p-quadrature
    mxp = np.ascontiguousarray(
        np.concatenate([Mx, Mp], axis=1)).astype(np.float16)   # [128, 256]

    dV = (S ** 2).sum(axis=1)                                  # [256]
    bias = ((dV[0::2] + dV[1::2]) / 4.0 - 0.5)                 # [128] f64
    return mxp, bias


def build_bass():
    nc = bacc.Bacc("TRN2", target_bir_lowering=False, debug=False,
                   num_devices=N_CORES)

    x_d = nc.dram_tensor("x", [128, ROWS], F16, kind="ExternalInput")
    mxp_d = nc.dram_tensor("mxp", [128, 256], F16, kind="ExternalInput")
    out_d = nc.dram_tensor("out", [128, ROWS], BF16, kind="ExternalOutput")
    x_ap = x_d.ap()
    out_ap = out_d.ap()

    Sq = mybir.ActivationFunctionType.Square

    with tile.TileContext(nc) as tc:
        with (
            tc.tile_pool(name="const", bufs=1) as const_pool,
            tc.tile_pool(name="xin", bufs=len(IN_PIECES)) as xin_pool,
            tc.tile_pool(name="oout", bufs=len(OUT_PIECES)) as oout_pool,
            tc.tile_pool(name="sqx", bufs=8) as sqx_pool,
            tc.tile_pool(name="psa", bufs=2, space="PSUM") as psa_pool,
            tc.tile_pool(name="psb", bufs=2, space="PSUM") as psb_pool,
        ):
            mxp_sb = const_pool.tile([128, 256], F16)

            # Input DMAs first, in program order on the SP HWDGE ring:
            # weights (64KB, needed by the first LDWEIGHTS) -> x pieces.
            # Ring FIFO keeps input bytes ahead of all output bytes.
            nc.sync.dma_start(out=mxp_sb, in_=mxp_d.ap())
            x_tiles = []          # (tile, start_col, cols)
            off = 0
            for k, cols in enumerate(IN_PIECES):
                t = xin_pool.tile([128, cols], F16, tag="x_sb",
                                  name=f"x_sb_{k}")
                nc.sync.dma_start(out=t, in_=x_ap[:, off:off + cols])
                x_tiles.append((t, off, cols))
                off += cols

            def locate(tiles, col):
                for t, start, cols in tiles:
                    if start <= col < start + cols:
                        return t, col - start
                raise AssertionError(col)

            o_tiles = []          # [tile, start_col, cols, pairs_done]
            ooff = 0
            for k, cols in enumerate(OUT_PIECES):
                t = oout_pool.tile([128, cols], BF16, tag="o_sb",
                                   name=f"o_sb_{k}")
                o_tiles.append([t, ooff, cols, 0])
                ooff += cols

            for g in range(PAIRS):
                cg = 1024 * g
                xt0, xo0 = locate(x_tiles, cg)          # chunk c0 cols
                xt1, xo1 = locate(x_tiles, cg + 512)    # chunk c1 cols
                # mux banks (psa, freed by ACT) and mup banks (psb, freed
                # by the DVE fused op) live in separate pools so psa
                # recycles early instead of riding out the whole
                # ACT -> DVE chain.
                psa = psa_pool.tile([128, 1024], F32)
                psb = psb_pool.tile([128, 1024], F32)
                nc.tensor.matmul(psa[:, 0:512], mxp_sb[:, 0:128],
                                 xt0[:, xo0:xo0 + 512], start=True, stop=True)
                nc.tensor.matmul(psa[:, 512:1024], mxp_sb[:, 0:128],
                                 xt1[:, xo1:xo1 + 512],
                                 start=True, stop=True)
                nc.tensor.matmul(psb[:, 0:512], mxp_sb[:, 128:256],
                                 xt0[:, xo0:xo0 + 512], start=True, stop=True)
                nc.tensor.matmul(psb[:, 512:1024], mxp_sb[:, 128:256],
                                 xt1[:, xo1:xo1 + 512],
                                 start=True, stop=True)

                sqx = sqx_pool.tile([128, 1024], BF16, tag="sqx",
                                    name=f"sqx_{g}")
                nc.scalar.activation(sqx, psa, Sq)

                # Write the fused result into the covering output piece(s);
                # the last pair spans two 512-col tail pieces.
                done = 0
                while done < 1024:
                    col = cg + done
                    rec = next(r for r in o_tiles
                               if r[1] <= col < r[1] + r[2])
                    ot, ostart, ocols, _ = rec
                    oo = col - ostart
                    seg = min(1024 - done, ostart + ocols - col)
                    nc.vector._custom_dve(
                        SQUARE_PLUS, out=ot[:, oo:oo + seg],
                        in0=psb[:, done:done + seg],
                        in1=sqx[:, done:done + seg])
                    rec[3] += seg
                    if rec[3] == ocols:
                        nc.sync.dma_start(
                            out=out_ap[:, ostart:ostart + ocols], in_=ot)
                    done += seg

    nc.compile()
    return nc


_NC_CACHE = None
_BIAS = None


def make_in_maps(X: np.ndarray, params: np.ndarray):
    global _BIAS
    mxp, bias = host_prep(params)
    _BIAS = bias.astype(np.float32)
    xt = np.ascontiguousarray(X.astype(np.float16).T)     # [128, BATCH]
    return [
        {"x": np.ascontiguousarray(xt[:, i * ROWS:(i + 1) * ROWS]),
         "mxp": mxp}
        for i in range(N_CORES)
    ]


def assemble_output(results) -> np.ndarray:
    full = np.concatenate([r["out"] for r in results], axis=1)  # [128, BATCH]
    out = full.T.astype(np.float32)
    out += _BIAS[None, :]
    return np.ascontiguousarray(out)


def _spot_check(out: np.ndarray, X: np.ndarray, mxp: np.ndarray) -> float:
    """Max rel err of the device output on 16 rows per core shard,
    recomputed on host with the same fp16 weights."""
    rows = np.concatenate([np.arange(i * ROWS, i * ROWS + 16)
                           for i in range(N_CORES)])
    mu = X[rows].astype(np.float32) @ mxp.astype(np.float32)
    ref = mu[:, :128] ** 2 + mu[:, 128:] ** 2 + _BIAS[None, :]
    err = np.abs(out[rows] - ref) / np.maximum(np.abs(ref), 1e-6)
    return float(err.max())


def kernel(**inputs: np.ndarray) -> np.ndarray:
    global _NC_CACHE
    X = np.asarray(inputs["inputs"], dtype=np.float32)
    params = np.asarray(inputs["params"], dtype=np.float32)
    assert X.shape == (BATCH, N_QUMODES)

    if _NC_CACHE is None:
        _NC_CACHE = build_bass()
    nc = _NC_CACHE

    in_maps = make_in_maps(X, params)
    out = None
    for _ in range(3):
        res = run_bass_kernel_spmd(nc, in_maps,
                                   core_ids=list(range(N_CORES)))
        out = assemble_output(res.results)
        # guards against a cold-start mis-execution (seen once on a
        # freshly loaded NEFF); healthy runs measure ~1e-2 here
        if _spot_check(out, X, in_maps[0]["mxp"]) < 0.05:
            break
    return out


# revision 16
# speedup vs baseline: 1.2412x; 1.0064x over previous
"""Trainium2 Bass kernel for the ContinuousVariableQNN problem (v4).

Math reduction (validated against the jax reference on host):
  The reference builds a 256x256 symplectic matrix S from params, then
    mu   = mu0 @ S.T   with mu0[:, 0::2] = 2*inputs (odd cols zero)
    n    = (dsum + mu_x^2 + mu_p^2) / (2*hbar) - 0.5
  Because mu0's p-quadrature entries are all zero, the big matmul collapses to
    mu_dev = inputs @ Ms          with Ms[i, j] = S[j, 2*i]   ([128, 256])
  (factor 2 from displacement and the 1/4 normalization cancel), and
    n[b, m] = mu_dev[b, 2m]^2 + mu_dev[b, 2m+1]^2 + bias[m]
  with bias[m] = (diag(S S^T)[2m] + diag(S S^T)[2m+1])/4 - 0.5 (a constant).

v4 device strategy (vs the 49.4us v2 baseline, from NTFF trace analysis):
  v2's wall was the Scalar engine squaring all four PSUM banks per pair
  (2.0us x 16 = 32us busy) plus a DVE combine pass.  The ISA allows at most
  ONE PSUM input per vector instruction, so the mup^2 + sqx combine cannot
  be a stock two-PSUM op -- instead v4 registers a custom DVE microcode op
  (the documented extension point in concourse.dve_ops):
      SQUARE_PLUS_ANT:  out = sq(Src0) + Src1      (1 uop, row 17)
  Per 1024-col pair (PSUM banks [mux0 mux1 mup0 mup1]):
      ACT: sqx = Square(mux banks)      PSUM->SBUF bf16   (1024+352)/1.2GHz
      DVE: out = sq(mup banks) + sqx    PSUM+SBUF->SBUF   ~(1024+240)/0.96GHz
  Two perfectly balanced passes (~1.15us / ~1.35us), zero extra combine.
  The +bias[m] is applied on the host during unshard (f32 broadcast add
  folded into the existing transpose+astype).  IO: all input pieces trigger
  first on the SP HWDGE ring (ring FIFO = input streams at line rate,
  outputs drain behind); 1024-col lead-in/tail pieces shorten the edges.
  Expected ~ring-bound: ~23.4us of HBM bytes + fixed framework overhead.
"""

import ml_dtypes
import numpy as np

import concourse.bass as bass
import concourse.mybir as mybir
import concourse.tile as tile
from concourse import bacc
from concourse import dve_ops as _D
from concourse.dve_spec import Spec, Src0, Src1, sq as _sq, lower as _lower
from concourse.dve_uop import DveOpSpec as _DveOpSpec
from concourse.bass_utils import run_bass_kernel_spmd

N_QUMODES = 128
N_LAYERS = 8
BATCH = 131072
N_CORES = 8
ROWS = BATCH // N_CORES          # 16384 batch columns per core (free dim)
PAIRS = 16                       # 1024-col work units
F32 = mybir.dt.float32
F16 = mybir.dt.float16
BF16 = mybir.dt.bfloat16

IN_PIECES = [512, 512, 1024, 2048] + [4096] * 3
OUT_PIECES = [1024, 1024] + [2048] * 6 + [1024, 512, 512]


def _register_square_plus():
    """out = sq(Src0) + Src1 via the custom-DVE extension point."""
    name = "SQUARE_PLUS_ANT"
    if name in _D._SUB_OPCODE_FOR_NAME:
        return next(op for op in _D.OPS if op.name == name)
    spec = Spec(
        body=_sq(Src0) + Src1,
        reference=lambda in0, in1, s0, s1, imm2:
            in0.astype(np.float32) ** 2 + in1,
    )
    row = max(_D._SUB_OPCODE_FOR_NAME.values()) + 1
    assert row < 0x20
    shas = {}
    for ver in ("v3",):
        s = _DveOpSpec(name=name, opcode=row, uops=_lower(spec, ver=ver),
                       rd1_en=_D.has_src1(spec))
        shas[ver] = s.sha(ver)
    op = _D.DveOp(name, spec, subdim=False, uops_sha=shas)
    _D.OPS.append(op)
    _D.CUSTOM_DVE_SPECS[name] = spec
    _D._SUB_OPCODE_FOR_NAME[name] = row
    return op


SQUARE_PLUS = _register_square_plus()


def host_prep(params: np.ndarray):
    """Build Mxp [128, 256] fp16 (Mx | Mp) and bias [128] f64 on host."""
    L, N = N_LAYERS, N_QUMODES
    p = params.reshape(L, N, 3).astype(np.float64)
    th1, r, th2 = p[..., 0], p[..., 1], p[..., 2]

    def rot(th):
        c, s = np.cos(th), np.sin(th)
        return np.stack([np.stack([c, -s], -1), np.stack([s, c], -1)], -2)

    z = np.zeros_like(r)
    sq = np.stack([np.stack([np.exp(-r), z], -1),
                   np.stack([z, np.exp(r)], -1)], -2)
    blk = np.einsum('lnab,lnbc,lncd->lnad', rot(th2), sq, rot(th1))

    t = np.cos(np.pi / 4)
    rr = np.sin(np.pi / 4)
    BS4 = np.array([[t, 0., -rr, 0.],
                    [0., t, 0., -rr],
                    [rr, 0., t, 0.],
                    [0., rr, 0., t]])
    C = np.eye(2 * N)
    for i in range(N - 1):
        C[2 * i:2 * i + 4, :] = BS4 @ C[2 * i:2 * i + 4, :]

    S = np.eye(2 * N)
    idx = np.arange(N)
    for l in range(L):
        D = np.zeros((N, 2, N, 2))
        D[idx, :, idx, :] = blk[l]
        S = C @ (D.reshape(2 * N, 2 * N) @ S)

    # mu_dev[b, j] = (inputs @ Ms)[b, j] with Ms = S[:, 0::2].T  [128, 256].
    Ms = S[:, 0::2].T
    Mx = Ms[:, 0::2]                 # [128 feat, 128 mode] x-quadrature
    Mp = Ms[:, 1::2]                 # p-quadrature
    mxp = np.ascontiguousarray(
        np.concatenate([Mx, Mp], axis=1)).astype(np.float16)   # [128, 256]

    dV = (S ** 2).sum(axis=1)                                  # [256]
    bias = ((dV[0::2] + dV[1::2]) / 4.0 - 0.5)                 # [128] f64
    return mxp, bias


def build_bass():
    nc = bacc.Bacc("TRN2", target_bir_lowering=False, debug=False,
                   num_devices=N_CORES)

    x_d = nc.dram_tensor("x", [128, ROWS], F16, kind="ExternalInput")
    mxp_d = nc.dram_tensor("mxp", [128, 256], F16, kind="ExternalInput")
    out_d = nc.dram_tensor("out", [128, ROWS], BF16, kind="ExternalOutput")
    x_ap = x_d.ap()
    out_ap = out_d.ap()

    Sq = mybir.ActivationFunctionType.Square

    with tile.TileContext(nc) as tc:
        with (
            tc.tile_pool(name="const", bufs=1) as const_pool,
            tc.tile_pool(name="xin", bufs=len(IN_PIECES)) as xin_pool,
            tc.tile_pool(name="oout", bufs=len(OUT_PIECES)) as oout_pool,
            tc.tile_pool(name="sqx", bufs=8) as sqx_pool,
            tc.tile_pool(name="psa", bufs=2, space="PSUM") as psa_pool,
            tc.tile_pool(name="psb", bufs=2, space="PSUM") as psb_pool,
        ):
            mxp_sb = const_pool.tile([128, 256], F16)

            # Input DMAs first, in program order on the SP HWDGE ring:
            # weights (64KB, needed by the first LDWEIGHTS) -> x pieces.
            # Ring FIFO keeps input bytes ahead of all output bytes.
            nc.sync.dma_start(out=mxp_sb, in_=mxp_d.ap())
            x_tiles = []          # (tile, start_col, cols)
            off = 0
            for k, cols in enumerate(IN_PIECES):
                t = xin_pool.tile([128, cols], F16, tag="x_sb",
                                  name=f"x_sb_{k}")
                nc.sync.dma_start(out=t, in_=x_ap[:, off:off + cols])
                x_tiles.append((t, off, cols))
                off += cols

            def locate(tiles, col):
                for t, start, cols in tiles:
                    if start <= col < start + cols:
                        return t, col - start
                raise AssertionError(col)

            o_tiles = []          # [tile, start_col, cols, pairs_done]
            ooff = 0
            for k, cols in enumerate(OUT_PIECES):
                t = oout_pool.tile([128, cols], BF16, tag="o_sb",
                                   name=f"o_sb_{k}")
                o_tiles.append([t, ooff, cols, 0])
                ooff += cols

            for g in range(PAIRS):
                cg = 1024 * g
                xt0, xo0 = locate(x_tiles, cg)          # chunk c0 cols
                xt1, xo1 = locate(x_tiles, cg + 512)    # chunk c1 cols
                # mux banks (psa, freed by ACT) and mup banks (psb, freed
                # by the DVE fused op) live in separate pools so psa
                # recycles early instead of riding out the whole
                # ACT -> DVE chain.
                psa = psa_pool.tile([128, 1024], F32)
                psb = psb_pool.tile([128, 1024], F32)
                nc.tensor.matmul(psa[:, 0:512], mxp_sb[:, 0:128],
                                 xt0[:, xo0:xo0 + 512], start=True, stop=True)
                nc.tensor.matmul(psa[:, 512:1024], mxp_sb[:, 0:128],
                                 xt1[:, xo1:xo1 + 512],
                                 start=True, stop=True)
                nc.tensor.matmul(psb[:, 0:512], mxp_sb[:, 128:256],
                                 xt0[:, xo0:xo0 + 512], start=True, stop=True)
                nc.tensor.matmul(psb[:, 512:1024], mxp_sb[:, 128:256],
                                 xt1[:, xo1:xo1 + 512],
                                 start=True, stop=True)

                sqx = sqx_pool.tile([128, 1024], BF16, tag="sqx",
                                    name=f"sqx_{g}")
                nc.scalar.activation(sqx, psa, Sq)

                # Write the fused result into the covering output piece(s);
                # the last pair spans two 512-col tail pieces.
                done = 0
                while done < 1024:
                    col = cg + done
                    rec = next(r for r in o_tiles
                               if r[1] <= col < r[1] + r[2])
                    ot, ostart, ocols, _ = rec
                    oo = col - ostart
                    seg = min(1024 - done, ostart + ocols - col)
                    nc.vector._custom_dve(
                        SQUARE_PLUS, out=ot[:, oo:oo + seg],
                        in0=psb[:, done:done + seg],
                        in1=sqx[:, done:done + seg])
                    rec[3] += seg
                    if rec[3] == ocols:
                        nc.sync.dma_start(
                            out=out_ap[:, ostart:ostart + ocols], in_=ot)
                    done += seg

    nc.compile()
    return nc


_NC_CACHE = None
_BIAS = None


def make_in_maps(X: np.ndarray, params: np.ndarray):
    global _BIAS
    mxp, bias = host_prep(params)
    _BIAS = bias.astype(np.float32)
    xt = np.ascontiguousarray(X.astype(np.float16).T)     # [128, BATCH]
    return [
        {"x": np.ascontiguousarray(xt[:, i * ROWS:(i + 1) * ROWS]),
         "mxp": mxp}
        for i in range(N_CORES)
    ]


def assemble_output(results) -> np.ndarray:
    full = np.concatenate([r["out"] for r in results], axis=1)  # [128, BATCH]
    out = full.T.astype(np.float32)
    out += _BIAS[None, :]
    return np.ascontiguousarray(out)


def _spot_check(out: np.ndarray, X: np.ndarray, mxp: np.ndarray) -> float:
    """Max rel err of the device output on 16 rows per core shard,
    recomputed on host with the same fp16 weights."""
    rows = np.concatenate([np.arange(i * ROWS, i * ROWS + 16)
                           for i in range(N_CORES)])
    mu = X[rows].astype(np.float32) @ mxp.astype(np.float32)
    ref = mu[:, :128] ** 2 + mu[:, 128:] ** 2 + _BIAS[None, :]
    err = np.abs(out[rows] - ref) / np.maximum(np.abs(ref), 1e-6)
    return float(err.max())


def kernel(**inputs: np.ndarray) -> np.ndarray:
    global _NC_CACHE
    X = np.asarray(inputs["inputs"], dtype=np.float32)
    params = np.asarray(inputs["params"], dtype=np.float32)
    assert X.shape == (BATCH, N_QUMODES)

    if _NC_CACHE is None:
        _NC_CACHE = build_bass()
    nc = _NC_CACHE

    in_maps = make_in_maps(X, params)
    out = None
    for _ in range(3):
        res = run_bass_kernel_spmd(nc, in_maps,
                                   core_ids=list(range(N_CORES)))
        out = assemble_output(res.results)
        # guards against a cold-start mis-execution (seen once on a
        # freshly loaded NEFF); healthy runs measure ~1e-2 here
        if _spot_check(out, X, in_maps[0]["mxp"]) < 0.05:
            break
    return out


# revision 17
# speedup vs baseline: 1.2671x; 1.0209x over previous
"""Trainium2 Bass kernel for the ContinuousVariableQNN problem (v4).

Math reduction (validated against the jax reference on host):
  The reference builds a 256x256 symplectic matrix S from params, then
    mu   = mu0 @ S.T   with mu0[:, 0::2] = 2*inputs (odd cols zero)
    n    = (dsum + mu_x^2 + mu_p^2) / (2*hbar) - 0.5
  Because mu0's p-quadrature entries are all zero, the big matmul collapses to
    mu_dev = inputs @ Ms          with Ms[i, j] = S[j, 2*i]   ([128, 256])
  (factor 2 from displacement and the 1/4 normalization cancel), and
    n[b, m] = mu_dev[b, 2m]^2 + mu_dev[b, 2m+1]^2 + bias[m]
  with bias[m] = (diag(S S^T)[2m] + diag(S S^T)[2m+1])/4 - 0.5 (a constant).

v4 device strategy (vs the 49.4us v2 baseline, from NTFF trace analysis):
  v2's wall was the Scalar engine squaring all four PSUM banks per pair
  (2.0us x 16 = 32us busy) plus a DVE combine pass.  The ISA allows at most
  ONE PSUM input per vector instruction, so the mup^2 + sqx combine cannot
  be a stock two-PSUM op -- instead v4 registers a custom DVE microcode op
  (the documented extension point in concourse.dve_ops):
      SQUARE_PLUS_ANT:  out = sq(Src0) + Src1      (1 uop, row 17)
  Per 1024-col pair (PSUM banks [mux0 mux1 mup0 mup1]):
      ACT: sqx = Square(mux banks)      PSUM->SBUF bf16   (1024+352)/1.2GHz
      DVE: out = sq(mup banks) + sqx    PSUM+SBUF->SBUF   ~(1024+240)/0.96GHz
  Two perfectly balanced passes (~1.15us / ~1.35us), zero extra combine.
  The +bias[m] is applied on the host during unshard (f32 broadcast add
  folded into the existing transpose+astype).  IO: all input pieces trigger
  first on the SP HWDGE ring (ring FIFO = input streams at line rate,
  outputs drain behind); 1024-col lead-in/tail pieces shorten the edges.
  Expected ~ring-bound: ~23.4us of HBM bytes + fixed framework overhead.
"""

import ml_dtypes
import numpy as np

import concourse.bass as bass
import concourse.mybir as mybir
import concourse.tile as tile
from concourse import bacc
from concourse import dve_ops as _D
from concourse.dve_spec import Spec, Src0, Src1, sq as _sq, lower as _lower
from concourse.dve_uop import DveOpSpec as _DveOpSpec
from concourse.bass_utils import run_bass_kernel_spmd

N_QUMODES = 128
N_LAYERS = 8
BATCH = 131072
N_CORES = 8
ROWS = BATCH // N_CORES          # 16384 batch columns per core (free dim)
PAIRS = 16                       # 1024-col work units
F32 = mybir.dt.float32
F16 = mybir.dt.float16
BF16 = mybir.dt.bfloat16

IN_PIECES = [512, 512, 1024] + [2048] * 7
OUT_PIECES = [1024, 1024] + [2048] * 6 + [1024, 1024]


def _register_square_plus():
    """out = sq(Src0) + Src1 via the custom-DVE extension point."""
    name = "SQUARE_PLUS_ANT"
    if name in _D._SUB_OPCODE_FOR_NAME:
        return next(op for op in _D.OPS if op.name == name)
    spec = Spec(
        body=_sq(Src0) + Src1,
        reference=lambda in0, in1, s0, s1, imm2:
            in0.astype(np.float32) ** 2 + in1,
    )
    row = max(_D._SUB_OPCODE_FOR_NAME.values()) + 1
    assert row < 0x20
    shas = {}
    for ver in ("v3",):
        s = _DveOpSpec(name=name, opcode=row, uops=_lower(spec, ver=ver),
                       rd1_en=_D.has_src1(spec))
        shas[ver] = s.sha(ver)
    op = _D.DveOp(name, spec, subdim=False, uops_sha=shas)
    _D.OPS.append(op)
    _D.CUSTOM_DVE_SPECS[name] = spec
    _D._SUB_OPCODE_FOR_NAME[name] = row
    return op


SQUARE_PLUS = _register_square_plus()


def host_prep(params: np.ndarray):
    """Build Mxp [128, 256] fp16 (Mx | Mp) and bias [128] f64 on host."""
    L, N = N_LAYERS, N_QUMODES
    p = params.reshape(L, N, 3).astype(np.float64)
    th1, r, th2 = p[..., 0], p[..., 1], p[..., 2]

    def rot(th):
        c, s = np.cos(th), np.sin(th)
        return np.stack([np.stack([c, -s], -1), np.stack([s, c], -1)], -2)

    z = np.zeros_like(r)
    sq = np.stack([np.stack([np.exp(-r), z], -1),
                   np.stack([z, np.exp(r)], -1)], -2)
    blk = np.einsum('lnab,lnbc,lncd->lnad', rot(th2), sq, rot(th1))

    t = np.cos(np.pi / 4)
    rr = np.sin(np.pi / 4)
    BS4 = np.array([[t, 0., -rr, 0.],
                    [0., t, 0., -rr],
                    [rr, 0., t, 0.],
                    [0., rr, 0., t]])
    C = np.eye(2 * N)
    for i in range(N - 1):
        C[2 * i:2 * i + 4, :] = BS4 @ C[2 * i:2 * i + 4, :]

    S = np.eye(2 * N)
    idx = np.arange(N)
    for l in range(L):
        D = np.zeros((N, 2, N, 2))
        D[idx, :, idx, :] = blk[l]
        S = C @ (D.reshape(2 * N, 2 * N) @ S)

    # mu_dev[b, j] = (inputs @ Ms)[b, j] with Ms = S[:, 0::2].T  [128, 256].
    Ms = S[:, 0::2].T
    Mx = Ms[:, 0::2]                 # [128 feat, 128 mode] x-quadrature
    Mp = Ms[:, 1::2]                 # p-quadrature
    mxp = np.ascontiguousarray(
        np.concatenate([Mx, Mp], axis=1)).astype(np.float16)   # [128, 256]

    dV = (S ** 2).sum(axis=1)                                  # [256]
    bias = ((dV[0::2] + dV[1::2]) / 4.0 - 0.5)                 # [128] f64
    return mxp, bias


def build_bass():
    nc = bacc.Bacc("TRN2", target_bir_lowering=False, debug=False,
                   num_devices=N_CORES)

    x_d = nc.dram_tensor("x", [128, ROWS], F16, kind="ExternalInput")
    mxp_d = nc.dram_tensor("mxp", [128, 256], F16, kind="ExternalInput")
    out_d = nc.dram_tensor("out", [128, ROWS], BF16, kind="ExternalOutput")
    x_ap = x_d.ap()
    out_ap = out_d.ap()

    Sq = mybir.ActivationFunctionType.Square

    with tile.TileContext(nc) as tc:
        with (
            tc.tile_pool(name="const", bufs=1) as const_pool,
            tc.tile_pool(name="xin", bufs=len(IN_PIECES)) as xin_pool,
            tc.tile_pool(name="oout", bufs=len(OUT_PIECES)) as oout_pool,
            tc.tile_pool(name="sqx", bufs=8) as sqx_pool,
            tc.tile_pool(name="psa", bufs=2, space="PSUM") as psa_pool,
            tc.tile_pool(name="psb", bufs=2, space="PSUM") as psb_pool,
        ):
            mxp_sb = const_pool.tile([128, 256], F16)

            # Input DMAs first, in program order on the SP HWDGE ring:
            # weights (64KB, needed by the first LDWEIGHTS) -> x pieces.
            # Ring FIFO keeps input bytes ahead of all output bytes.
            nc.sync.dma_start(out=mxp_sb, in_=mxp_d.ap())
            x_tiles = []          # (tile, start_col, cols)
            off = 0
            for k, cols in enumerate(IN_PIECES):
                t = xin_pool.tile([128, cols], F16, tag="x_sb",
                                  name=f"x_sb_{k}")
                nc.sync.dma_start(out=t, in_=x_ap[:, off:off + cols])
                x_tiles.append((t, off, cols))
                off += cols

            def locate(tiles, col):
                for t, start, cols in tiles:
                    if start <= col < start + cols:
                        return t, col - start
                raise AssertionError(col)

            o_tiles = []          # [tile, start_col, cols, pairs_done]
            ooff = 0
            for k, cols in enumerate(OUT_PIECES):
                t = oout_pool.tile([128, cols], BF16, tag="o_sb",
                                   name=f"o_sb_{k}")
                o_tiles.append([t, ooff, cols, 0])
                ooff += cols

            for g in range(PAIRS):
                cg = 1024 * g
                xt0, xo0 = locate(x_tiles, cg)          # chunk c0 cols
                xt1, xo1 = locate(x_tiles, cg + 512)    # chunk c1 cols
                # mux banks (psa, freed by ACT) and mup banks (psb, freed
                # by the DVE fused op) live in separate pools so psa
                # recycles early instead of riding out the whole
                # ACT -> DVE chain.
                psa = psa_pool.tile([128, 1024], F32)
                psb = psb_pool.tile([128, 1024], F32)
                nc.tensor.matmul(psa[:, 0:512], mxp_sb[:, 0:128],
                                 xt0[:, xo0:xo0 + 512], start=True, stop=True)
                nc.tensor.matmul(psa[:, 512:1024], mxp_sb[:, 0:128],
                                 xt1[:, xo1:xo1 + 512],
                                 start=True, stop=True)
                nc.tensor.matmul(psb[:, 0:512], mxp_sb[:, 128:256],
                                 xt0[:, xo0:xo0 + 512], start=True, stop=True)
                nc.tensor.matmul(psb[:, 512:1024], mxp_sb[:, 128:256],
                                 xt1[:, xo1:xo1 + 512],
                                 start=True, stop=True)

                sqx = sqx_pool.tile([128, 1024], BF16, tag="sqx",
                                    name=f"sqx_{g}")
                nc.scalar.activation(sqx, psa, Sq)

                # Write the fused result into the covering output piece(s);
                # the last pair spans two 512-col tail pieces.
                done = 0
                while done < 1024:
                    col = cg + done
                    rec = next(r for r in o_tiles
                               if r[1] <= col < r[1] + r[2])
                    ot, ostart, ocols, _ = rec
                    oo = col - ostart
                    seg = min(1024 - done, ostart + ocols - col)
                    nc.vector._custom_dve(
                        SQUARE_PLUS, out=ot[:, oo:oo + seg],
                        in0=psb[:, done:done + seg],
                        in1=sqx[:, done:done + seg])
                    rec[3] += seg
                    if rec[3] == ocols:
                        nc.sync.dma_start(
                            out=out_ap[:, ostart:ostart + ocols], in_=ot)
                    done += seg

    nc.compile()
    return nc


_NC_CACHE = None
_BIAS = None


def make_in_maps(X: np.ndarray, params: np.ndarray):
    global _BIAS
    mxp, bias = host_prep(params)
    _BIAS = bias.astype(np.float32)
    xt = np.ascontiguousarray(X.astype(np.float16).T)     # [128, BATCH]
    return [
        {"x": np.ascontiguousarray(xt[:, i * ROWS:(i + 1) * ROWS]),
         "mxp": mxp}
        for i in range(N_CORES)
    ]


def assemble_output(results) -> np.ndarray:
    full = np.concatenate([r["out"] for r in results], axis=1)  # [128, BATCH]
    out = full.T.astype(np.float32)
    out += _BIAS[None, :]
    return np.ascontiguousarray(out)


def _spot_check(out: np.ndarray, X: np.ndarray, mxp: np.ndarray) -> float:
    """Max rel err of the device output on 16 rows per core shard,
    recomputed on host with the same fp16 weights."""
    rows = np.concatenate([np.arange(i * ROWS, i * ROWS + 16)
                           for i in range(N_CORES)])
    mu = X[rows].astype(np.float32) @ mxp.astype(np.float32)
    ref = mu[:, :128] ** 2 + mu[:, 128:] ** 2 + _BIAS[None, :]
    err = np.abs(out[rows] - ref) / np.maximum(np.abs(ref), 1e-6)
    return float(err.max())


def kernel(**inputs: np.ndarray) -> np.ndarray:
    global _NC_CACHE
    X = np.asarray(inputs["inputs"], dtype=np.float32)
    params = np.asarray(inputs["params"], dtype=np.float32)
    assert X.shape == (BATCH, N_QUMODES)

    if _NC_CACHE is None:
        _NC_CACHE = build_bass()
    nc = _NC_CACHE

    in_maps = make_in_maps(X, params)
    out = None
    for _ in range(3):
        res = run_bass_kernel_spmd(nc, in_maps,
                                   core_ids=list(range(N_CORES)))
        out = assemble_output(res.results)
        # guards against a cold-start mis-execution (seen once on a
        # freshly loaded NEFF); healthy runs measure ~1e-2 here
        if _spot_check(out, X, in_maps[0]["mxp"]) < 0.05:
            break
    return out
